# revision 14
# baseline (speedup 1.0000x reference)
"""Trainium2 Bass kernel for CoordLSVotingWeighted (segment_reduce).

Strategy: data-parallel over batch B=8 across 8 NeuronCores (1 image/core).
Host prep (per image): de-interleave `direct` into unit nx/ny (bf16),
transpose w (bf16) and seg (fp16) to channel-major [H, C, W] layouts.
Device per image:
  - softplus(w) on ScalarE: sp = Ln(1 + Exp(w))
  - hard one-hot of argmax over 9 seg channels via DVE max-tree + is_equal
  - lhs features L = {hot, hot*ch, hot*cw} (bf16), rhs features
    F = {sp, R11=sp*nx^2, m=sp*nx*ny} (bf16; R00 = sp - R11 recovered on host)
  - segment reduce via 32 accumulating TensorE matmuls, 4 w-columns per
    matmul packed block-diagonally: lhsT [128, 4x32] (FWL-sized 128 cols),
    rhs [128, 4x27] -> PSUM [128, 108]; host sums the 4 diagonal blocks.
  - PE warm-up matmuls on junk data keep the HAM clock gate at full rate.
Host: assemble 2x2 systems in float64, pinv-solve, scale by HEIGHT.

Self-contained: only needs numpy / ml_dtypes / concourse (installed env).
"""

import os

import numpy as np

B = 8
H = 128
W = 128
NCLS = 9  # seg channels, class 0 = background
NPTS = 9
OC = 8
HEIGHT = 128.0
N_CORES = 8

NF = W * NPTS  # 1152
GJ = 4  # w-columns per matmul (block-diagonal packing)
LF = 32  # lhs feature rows (24 real + 8 pad) -> GJ*LF = 128 weight cols (FWL)
RF = 27  # rhs feature rows -> GJ*RF = 108 psum cols
N_MM = W // GJ  # 32 accumulating matmuls
N_WARM_A = 48  # junk matmuls during the DMA window (HAM clock ramp)
N_WARM_B = 10  # junk matmuls gated on softplus output (keep HAM hot)

_cache: dict = {}


def _build_nc():
    import concourse.bacc as bacc
    import concourse.tile as tile
    import concourse.mybir as mybir
    from concourse.alu_op_type import AluOpType as Alu

    Act = mybir.ActivationFunctionType
    f32 = mybir.dt.float32
    f16 = mybir.dt.float16
    b16 = mybir.dt.bfloat16

    nc = bacc.Bacc(
        "TRN2", target_bir_lowering=False, debug=False, num_devices=N_CORES
    )
    seg_d = nc.dram_tensor("seg16", [H, NCLS * W], f16, kind="ExternalInput")
    nx_d = nc.dram_tensor("nx", [H, NF], b16, kind="ExternalInput")
    ny_d = nc.dram_tensor("ny", [H, NF], b16, kind="ExternalInput")
    w_d = nc.dram_tensor("wgt", [H, NF], b16, kind="ExternalInput")
    cw_d = nc.dram_tensor("cwv", [H, W], b16, kind="ExternalInput")
    ch_d = nc.dram_tensor("chv", [H, 1], f32, kind="ExternalInput")
    out_d = nc.dram_tensor("acc", [GJ * RF, GJ * LF], f32, kind="ExternalOutput")

    with tile.TileContext(nc) as tc:
        with (
            tc.tile_pool(name="main", bufs=1) as pool,
            tc.tile_pool(name="ps", bufs=1, space="PSUM") as psp,
        ):
            segt = pool.tile([H, NCLS * W], f16, tag="segt")
            nxt = pool.tile([H, NF], b16, tag="nxt")
            nyt = pool.tile([H, NF], b16, tag="nyt")
            wt = pool.tile([H, NF], b16, tag="wt")
            cwt = pool.tile([H, W], b16, tag="cwt")
            cht = pool.tile([H, 1], f32, tag="cht")
            warm = pool.tile([H, W], b16, tag="warm")
            ew = pool.tile([H, NF], b16, tag="ew")
            ut = pool.tile([H, NF], b16, tag="ut")
            tmx = pool.tile([H, 8 * W], f16, tag="tmx")
            L = pool.tile([H, LF * W], b16, tag="L")
            R = pool.tile([H, RF * W], b16, tag="R")
            outs = pool.tile([GJ * RF, GJ * LF], f32, tag="outs")

            acc = psp.tile([GJ * RF, GJ * LF], f32, tag="acc")
            pwarm = psp.tile([GJ * RF, GJ * LF], f32, tag="pwarm")

            # ---- all input DMAs on one queue, in priority order: the DGE
            # rings drain descriptors in dispatch order, so w (softplus
            # critical path) completes first, then seg (one-hot), then nx/ny.
            nc.sync.dma_start(out=wt[:, :], in_=w_d[:, :])
            nc.sync.dma_start(out=segt[:, :], in_=seg_d[:, :])
            nc.sync.dma_start(out=nxt[:, :], in_=nx_d[:, :])
            nc.sync.dma_start(out=nyt[:, :], in_=ny_d[:, :])
            nc.sync.dma_start(out=cwt[:, :], in_=cw_d[:, :])
            nc.sync.dma_start(out=cht[:, :], in_=ch_d[:, :])
            nc.gpsimd.memset(warm[:, :], 0.0)
            # zero the 8 pad feature rows of L (f = 24..31)
            nc.gpsimd.memset(L[:, 24 * W : 32 * W], 0.0)

            # ---- PE warm-up A: junk matmuls during the DMA window
            for _ in range(N_WARM_A):
                nc.tensor.matmul(
                    pwarm[:, :], warm[:, 0 : GJ * RF], warm[:, 0 : GJ * LF],
                    start=True, stop=True,
                )

            # ---- softplus on ScalarE: sp = Ln(1 + Exp(w)) -> R rows 0..8
            nc.scalar.activation(out=ew[:, :], in_=wt[:, :], func=Act.Exp)
            nc.scalar.activation(
                out=R[:, 0:NF], in_=ew[:, :], func=Act.Ln, bias=1.0
            )

            # ---- one-hot via DVE max-tree (channel-major fp16) + is_equal
            t1 = tmx[:, 0 : 4 * W]
            t2 = tmx[:, 4 * W : 6 * W]
            t3 = tmx[:, 6 * W : 7 * W]
            mx = tmx[:, 7 * W : 8 * W]
            nc.vector.tensor_tensor(
                out=t1, in0=segt[:, W : 5 * W], in1=segt[:, 5 * W : 9 * W],
                op=Alu.max,
            )
            nc.vector.tensor_tensor(
                out=t2, in0=t1[:, 0 : 2 * W], in1=t1[:, 2 * W : 4 * W], op=Alu.max
            )
            nc.vector.tensor_tensor(
                out=t3, in0=t2[:, 0:W], in1=t2[:, W : 2 * W], op=Alu.max
            )
            nc.vector.tensor_tensor(
                out=mx, in0=t3, in1=segt[:, 0:W], op=Alu.max
            )
            seg_fg = segt[:, W : 9 * W].rearrange("q (c w) -> q c w", c=OC)
            mx_b = mx.unsqueeze(1).broadcast_to((H, OC, W))
            hot_r = L[:, 0 : OC * W].rearrange("q (c w) -> q c w", c=OC)
            nc.vector.tensor_tensor(
                out=hot_r, in0=seg_fg, in1=mx_b, op=Alu.is_equal
            )
            # hot*ch (per-partition scalar), hot*cw (broadcast along c)
            nc.vector.tensor_scalar_mul(
                L[:, OC * W : 2 * OC * W], L[:, 0 : OC * W], cht[:, :]
            )
            cw_b = cwt[:, :].unsqueeze(1).broadcast_to((H, OC, W))
            nc.vector.tensor_tensor(
                out=L[:, 2 * OC * W : 3 * OC * W].rearrange(
                    "q (c w) -> q c w", c=OC
                ),
                in0=L[:, 0 : OC * W].rearrange("q (c w) -> q c w", c=OC),
                in1=cw_b, op=Alu.mult,
            )

            # ---- PE warm-up B: gated on softplus output, keeps HAM hot
            for _ in range(N_WARM_B):
                nc.tensor.matmul(
                    pwarm[:, :], R[:, 0 : GJ * RF], warm[:, 0 : GJ * LF],
                    start=True, stop=True,
                )

            # ---- rhs features: u = sp*nx, R11 = u*nx, m = u*ny
            nc.vector.tensor_tensor(
                out=ut[:, :], in0=R[:, 0:NF], in1=nxt[:, :], op=Alu.mult
            )
            nc.vector.tensor_tensor(
                out=R[:, NF : 2 * NF], in0=ut[:, :], in1=nxt[:, :], op=Alu.mult
            )
            nc.vector.tensor_tensor(
                out=R[:, 2 * NF : 3 * NF], in0=ut[:, :], in1=nyt[:, :], op=Alu.mult
            )

            # ---- segment reduce: 32 accumulating matmuls, 4 w-cols each,
            # packed block-diagonally via flat stride-32 slices:
            # stationary R[:, i::32] cols c=4g+t <-> (g, w=i+32t), 108 cols;
            # moving L[:, i::32] cols c'=4f+t' <-> (f, w=i+32t'), 128 cols.
            # psum[c, c'] valid where t == t'; host sums the diagonal.
            for i in range(N_MM):
                nc.tensor.matmul(
                    acc[:, :],
                    R[:, i::N_MM],
                    L[:, i::N_MM],
                    start=(i == 0),
                    stop=(i == N_MM - 1),
                )

            nc.scalar.copy(out=outs[:, :], in_=acc[:, :])
            nc.sync.dma_start(out=out_d[:, :], in_=outs[:, :])

    nc.compile()
    return nc


def _host_inputs(seg, direct, w):
    import ml_dtypes

    bf16 = ml_dtypes.bfloat16
    # unit direction vectors (divide_no_nan semantics)
    n = direct.reshape(B, H, W, NPTS, 2).astype(np.float32)
    norm = np.sqrt(n[..., 0] ** 2 + n[..., 1] ** 2)
    safe = np.where(norm == 0.0, 1.0, norm)
    nx = np.where(norm == 0.0, 0.0, n[..., 0] / safe)
    ny = np.where(norm == 0.0, 0.0, n[..., 1] / safe)
    # [B, H, W, C] -> channel-major per-row [B, H, C, W] contiguous
    seg16 = np.ascontiguousarray(seg.transpose(0, 1, 3, 2)).astype(np.float16)
    nx16 = np.ascontiguousarray(nx.transpose(0, 1, 3, 2)).astype(bf16)
    ny16 = np.ascontiguousarray(ny.transpose(0, 1, 3, 2)).astype(bf16)
    w16 = np.ascontiguousarray(w.transpose(0, 1, 3, 2)).astype(bf16)
    coord = ((np.arange(W, dtype=np.float32) + 0.5) / HEIGHT).astype(bf16)
    cwv = np.ascontiguousarray(np.broadcast_to(coord[None, :], (H, W)))
    chv = ((np.arange(H, dtype=np.float32) + 0.5) / HEIGHT).reshape(H, 1)
    return seg16, nx16, ny16, w16, cwv, chv


def _solve_host(acc_raw: np.ndarray) -> np.ndarray:
    """acc [108,128] fp32 -> p [OC, NPTS, 2] fp32 (float64 pinv like ref)."""
    x = acc_raw.astype(np.float64).reshape(RF, GJ, LF, GJ)
    a = np.einsum("gtft->fg", x)  # sum the GJ diagonal blocks -> [32, 27]
    A_sp = a[0:OC, 0:NPTS]
    A_r11 = a[0:OC, NPTS : 2 * NPTS]
    A_m = a[0:OC, 2 * NPTS : 3 * NPTS]
    C_sp = a[OC : 2 * OC, 0:NPTS]
    C_r11 = a[OC : 2 * OC, NPTS : 2 * NPTS]
    C_m = a[OC : 2 * OC, 2 * NPTS : 3 * NPTS]
    W_r11 = a[2 * OC : 3 * OC, NPTS : 2 * NPTS]
    W_m = a[2 * OC : 3 * OC, 2 * NPTS : 3 * NPTS]
    Rm = np.empty((OC, NPTS, 2, 2), dtype=np.float64)
    Rm[..., 0, 0] = A_sp - A_r11
    Rm[..., 0, 1] = -A_m
    Rm[..., 1, 0] = -A_m
    Rm[..., 1, 1] = A_r11
    q = np.stack([(C_sp - C_r11) - W_m, W_r11 - C_m], axis=-1)
    Rp = np.linalg.pinv(Rm.reshape(-1, 2, 2)).reshape(Rm.shape)
    p = np.einsum("cpij,cpj->cpi", Rp, q) * HEIGHT
    return p.astype(np.float32)


def kernel(seg, direct, w):
    if "nc" not in _cache:
        _cache["nc"] = _build_nc()
    nc = _cache["nc"]

    seg = np.asarray(seg, dtype=np.float32)
    direct = np.ascontiguousarray(np.asarray(direct, dtype=np.float32))
    w = np.asarray(w, dtype=np.float32)
    seg16, nx16, ny16, w16, cwv, chv = _host_inputs(seg, direct, w)

    in_maps = []
    for i in range(B):
        in_maps.append(
            {
                "seg16": seg16[i].reshape(H, NCLS * W),
                "nx": nx16[i].reshape(H, NF),
                "ny": ny16[i].reshape(H, NF),
                "wgt": w16[i].reshape(H, NF),
                "cwv": cwv,
                "chv": chv,
            }
        )

    from concourse.bass_utils import run_bass_kernel_spmd

    trace = bool(int(os.environ.get("KERNEL_TRACE", "0")))
    res = run_bass_kernel_spmd(
        nc, in_maps, core_ids=list(range(N_CORES)), trace=trace
    )
    kernel._last_exec_ns = res.exec_time_ns
    kernel._last_results = res

    out = np.stack(
        [_solve_host(np.asarray(res.results[i]["acc"])) for i in range(B)], axis=0
    )
    return out


# revision 27
# speedup vs baseline: 1.0739x; 1.0739x over previous
"""Trainium2 Bass kernel for CoordLSVotingWeighted (segment_reduce).

Strategy: data-parallel over batch B=8 across 8 NeuronCores (1 image/core).
Host prep (per image): de-interleave `direct` into unit nx/ny (bf16),
transpose w (bf16) and seg (fp16) to channel-major [H, C, W] layouts.
Device per image:
  - softplus(w) on ScalarE: sp = Ln(1 + Exp(w))
  - hard one-hot of argmax over 9 seg channels via DVE max-tree + is_equal
  - lhs features L = {hot, hot*ch, hot*cw} (bf16), rhs features
    F = {sp, R11=sp*nx^2, m=sp*nx*ny} (bf16; R00 = sp - R11 recovered on host)
  - segment reduce via 32 accumulating TensorE matmuls, 4 w-columns per
    matmul packed block-diagonally: lhsT [128, 4x32] (FWL-sized 128 cols),
    rhs [128, 4x27] -> PSUM [128, 108]; host sums the 4 diagonal blocks.
  - PE warm-up matmuls on junk data keep the HAM clock gate at full rate.
Host: assemble 2x2 systems in float64, pinv-solve, scale by HEIGHT.

Self-contained: only needs numpy / ml_dtypes / concourse (installed env).
"""

import os

import numpy as np

B = 8
H = 128
W = 128
NCLS = 9  # seg channels, class 0 = background
NPTS = 9
OC = 8
HEIGHT = 128.0
N_CORES = 8

NF = W * NPTS  # 1152
GJ = 4  # w-columns per matmul (block-diagonal packing)
LF = 32  # lhs feature rows (24 real + 8 pad) -> GJ*LF = 128 weight cols (FWL)
RF = 27  # rhs feature rows -> GJ*RF = 108 psum cols
N_MM = W // GJ  # 32 accumulating matmuls
N_WARM_A = 48  # junk matmuls during the DMA window (HAM clock ramp)
N_WARM_B = 10  # junk matmuls gated on softplus output (keep HAM hot)

_cache: dict = {}


def _build_nc():
    import concourse.bacc as bacc
    import concourse.tile as tile
    import concourse.mybir as mybir
    from concourse.alu_op_type import AluOpType as Alu

    Act = mybir.ActivationFunctionType
    f32 = mybir.dt.float32
    f16 = mybir.dt.float16
    b16 = mybir.dt.bfloat16

    nc = bacc.Bacc(
        "TRN2", target_bir_lowering=False, debug=False, num_devices=N_CORES
    )
    seg_d = nc.dram_tensor("seg16", [H, NCLS * W], f16, kind="ExternalInput")
    nx_d = nc.dram_tensor("nx", [H, NF], b16, kind="ExternalInput")
    ny_d = nc.dram_tensor("ny", [H, NF], b16, kind="ExternalInput")
    w_d = nc.dram_tensor("wgt", [H, NF], b16, kind="ExternalInput")
    cw_d = nc.dram_tensor("cwv", [H, W], b16, kind="ExternalInput")
    ch_d = nc.dram_tensor("chv", [H, 1], f32, kind="ExternalInput")
    out_d = nc.dram_tensor("acc", [GJ * RF, GJ * LF], f32, kind="ExternalOutput")

    with tile.TileContext(nc) as tc:
        with (
            tc.tile_pool(name="main", bufs=1) as pool,
            tc.tile_pool(name="ps", bufs=1, space="PSUM") as psp,
        ):
            segt = pool.tile([H, NCLS * W], f16, tag="segt")
            nxt = pool.tile([H, NF], b16, tag="nxt")
            nyt = pool.tile([H, NF], b16, tag="nyt")
            wt = pool.tile([H, NF], b16, tag="wt")
            cwt = pool.tile([H, W], b16, tag="cwt")
            cht = pool.tile([H, 1], f32, tag="cht")
            warm = pool.tile([H, W], b16, tag="warm")
            ew = pool.tile([H, NF], b16, tag="ew")
            ut = pool.tile([H, NF], b16, tag="ut")
            tmx = pool.tile([H, 8 * W], f16, tag="tmx")
            L = pool.tile([H, LF * W], b16, tag="L")
            R = pool.tile([H, RF * W], b16, tag="R")
            outs = pool.tile([GJ * RF, GJ * LF], f32, tag="outs")

            acc = psp.tile([GJ * RF, GJ * LF], f32, tag="acc")
            pwarm = psp.tile([GJ * RF, GJ * LF], f32, tag="pwarm")

            # ---- DMA dispatches split across two queues so the transfers
            # start early; w (softplus critical path) leads the sync queue,
            # seg (one-hot path) leads the gpsimd queue.
            nc.sync.dma_start(out=wt[:, :], in_=w_d[:, :])
            nc.sync.dma_start(out=cwt[:, :], in_=cw_d[:, :])
            nc.scalar.dma_start(out=cht[:, :], in_=ch_d[:, :])
            nc.gpsimd.dma_start(out=segt[:, :], in_=seg_d[:, :])
            nc.gpsimd.dma_start(out=nxt[:, :], in_=nx_d[:, :])
            nc.gpsimd.dma_start(out=nyt[:, :], in_=ny_d[:, :])
            nc.gpsimd.memset(warm[:, :], 0.0)
            # zero the 8 pad feature rows of L (f = 24..31)
            nc.gpsimd.memset(L[:, 24 * W : 32 * W], 0.0)

            # ---- PE warm-up A: junk matmuls during the DMA window
            for _ in range(N_WARM_A):
                nc.tensor.matmul(
                    pwarm[:, :], warm[:, 0 : GJ * RF], warm[:, 0 : GJ * LF],
                    start=True, stop=True,
                )

            # ---- softplus on ScalarE: sp = Ln(1 + Exp(w)) -> R rows 0..8
            nc.scalar.activation(out=ew[:, :], in_=wt[:, :], func=Act.Exp)
            nc.scalar.activation(
                out=R[:, 0:NF], in_=ew[:, :], func=Act.Ln, bias=1.0
            )

            # ---- one-hot via DVE max-tree (channel-major fp16) + is_equal
            t1 = tmx[:, 0 : 4 * W]
            t2 = tmx[:, 4 * W : 6 * W]
            t3 = tmx[:, 6 * W : 7 * W]
            mx = tmx[:, 7 * W : 8 * W]
            nc.vector.tensor_tensor(
                out=t1, in0=segt[:, W : 5 * W], in1=segt[:, 5 * W : 9 * W],
                op=Alu.max,
            )
            nc.vector.tensor_tensor(
                out=t2, in0=t1[:, 0 : 2 * W], in1=t1[:, 2 * W : 4 * W], op=Alu.max
            )
            nc.vector.tensor_tensor(
                out=t3, in0=t2[:, 0:W], in1=t2[:, W : 2 * W], op=Alu.max
            )
            nc.vector.tensor_tensor(
                out=mx, in0=t3, in1=segt[:, 0:W], op=Alu.max
            )
            seg_fg = segt[:, W : 9 * W].rearrange("q (c w) -> q c w", c=OC)
            mx_b = mx.unsqueeze(1).broadcast_to((H, OC, W))
            hot_r = L[:, 0 : OC * W].rearrange("q (c w) -> q c w", c=OC)
            nc.vector.tensor_tensor(
                out=hot_r, in0=seg_fg, in1=mx_b, op=Alu.is_equal
            )
            # hot*ch (per-partition scalar), hot*cw (broadcast along c)
            nc.vector.tensor_scalar_mul(
                L[:, OC * W : 2 * OC * W], L[:, 0 : OC * W], cht[:, :]
            )
            cw_b = cwt[:, 0:W].unsqueeze(1).broadcast_to((H, OC, W))
            nc.vector.tensor_tensor(
                out=L[:, 2 * OC * W : 3 * OC * W].rearrange(
                    "q (c w) -> q c w", c=OC
                ),
                in0=L[:, 0 : OC * W].rearrange("q (c w) -> q c w", c=OC),
                in1=cw_b, op=Alu.mult,
            )

            # ---- PE warm-up B: gated on softplus output, keeps HAM hot
            for _ in range(N_WARM_B):
                nc.tensor.matmul(
                    pwarm[:, :], R[:, 0 : GJ * RF], warm[:, 0 : GJ * LF],
                    start=True, stop=True,
                )

            # ---- rhs features: u = sp*nx, R11 = u*nx, m = u*ny
            nc.vector.tensor_tensor(
                out=ut[:, :], in0=R[:, 0:NF], in1=nxt[:, :], op=Alu.mult
            )
            nc.vector.tensor_tensor(
                out=R[:, NF : 2 * NF], in0=ut[:, :], in1=nxt[:, :], op=Alu.mult
            )
            nc.vector.tensor_tensor(
                out=R[:, 2 * NF : 3 * NF], in0=ut[:, :], in1=nyt[:, :], op=Alu.mult
            )

            # ---- segment reduce: 32 accumulating matmuls, 4 w-cols each,
            # packed block-diagonally via flat stride-32 slices:
            # stationary R[:, i::32] cols c=4g+t <-> (g, w=i+32t), 108 cols;
            # moving L[:, i::32] cols c'=4f+t' <-> (f, w=i+32t'), 128 cols.
            # psum[c, c'] valid where t == t'; host sums the diagonal.
            for i in range(N_MM):
                nc.tensor.matmul(
                    acc[:, :],
                    R[:, i::N_MM],
                    L[:, i::N_MM],
                    start=(i == 0),
                    stop=(i == N_MM - 1),
                )

            nc.scalar.copy(out=outs[:, :], in_=acc[:, :])
            nc.sync.dma_start(out=out_d[:, :], in_=outs[:, :])

    nc.compile()

    # Post-compile: both Exp and Ln live in the natural_log_exp_and_others
    # ACT table set (index 6 in act_info.json), but the table-load pass
    # assigns each function its first containing set, inserting two loads.
    # Point the first load at the shared set and drop the rest (~1.3us each
    # off the critical path; walrus adopts pre-placed loads).
    NL_EXP_SET = 6
    loads = []
    for blk in nc.main_func.blocks:
        for ins in blk.instructions:
            if isinstance(ins, mybir.InstLoadActFuncSet):
                loads.append((blk, ins))
    assert len(loads) >= 1
    loads[0][1].act_func_set_id = NL_EXP_SET
    for blk, ins in loads[1:]:
        blk.instructions.remove(ins)
    return nc


def _host_inputs(seg, direct, w):
    import ml_dtypes

    bf16 = ml_dtypes.bfloat16
    # unit direction vectors (divide_no_nan semantics)
    n = direct.reshape(B, H, W, NPTS, 2).astype(np.float32)
    norm = np.sqrt(n[..., 0] ** 2 + n[..., 1] ** 2)
    safe = np.where(norm == 0.0, 1.0, norm)
    nx = np.where(norm == 0.0, 0.0, n[..., 0] / safe)
    ny = np.where(norm == 0.0, 0.0, n[..., 1] / safe)
    # [B, H, W, C] -> channel-major per-row [B, H, C, W] contiguous
    seg16 = np.ascontiguousarray(seg.transpose(0, 1, 3, 2)).astype(np.float16)
    nx16 = np.ascontiguousarray(nx.transpose(0, 1, 3, 2)).astype(bf16)
    ny16 = np.ascontiguousarray(ny.transpose(0, 1, 3, 2)).astype(bf16)
    w16 = np.ascontiguousarray(w.transpose(0, 1, 3, 2)).astype(bf16)
    coord = (np.arange(W, dtype=np.float32) + 0.5) / HEIGHT
    cwv = np.ascontiguousarray(
        np.broadcast_to(coord.astype(bf16)[None, :], (H, W))
    )
    chv = coord.reshape(H, 1).copy()
    return seg16, nx16, ny16, w16, cwv, chv


def _solve_host(acc_raw: np.ndarray) -> np.ndarray:
    """acc [108,128] fp32 -> p [OC, NPTS, 2] fp32 (float64 pinv like ref)."""
    x = acc_raw.astype(np.float64).reshape(RF, GJ, LF, GJ)
    a = np.einsum("gtft->fg", x)  # sum the GJ diagonal blocks -> [32, 27]
    A_sp = a[0:OC, 0:NPTS]
    A_r11 = a[0:OC, NPTS : 2 * NPTS]
    A_m = a[0:OC, 2 * NPTS : 3 * NPTS]
    C_sp = a[OC : 2 * OC, 0:NPTS]
    C_r11 = a[OC : 2 * OC, NPTS : 2 * NPTS]
    C_m = a[OC : 2 * OC, 2 * NPTS : 3 * NPTS]
    W_r11 = a[2 * OC : 3 * OC, NPTS : 2 * NPTS]
    W_m = a[2 * OC : 3 * OC, 2 * NPTS : 3 * NPTS]
    Rm = np.empty((OC, NPTS, 2, 2), dtype=np.float64)
    Rm[..., 0, 0] = A_sp - A_r11
    Rm[..., 0, 1] = -A_m
    Rm[..., 1, 0] = -A_m
    Rm[..., 1, 1] = A_r11
    q = np.stack([(C_sp - C_r11) - W_m, W_r11 - C_m], axis=-1)
    Rp = np.linalg.pinv(Rm.reshape(-1, 2, 2)).reshape(Rm.shape)
    p = np.einsum("cpij,cpj->cpi", Rp, q) * HEIGHT
    return p.astype(np.float32)


def kernel(seg, direct, w):
    if "nc" not in _cache:
        _cache["nc"] = _build_nc()
    nc = _cache["nc"]

    seg = np.asarray(seg, dtype=np.float32)
    direct = np.ascontiguousarray(np.asarray(direct, dtype=np.float32))
    w = np.asarray(w, dtype=np.float32)
    seg16, nx16, ny16, w16, cwv, chv = _host_inputs(seg, direct, w)

    in_maps = []
    for i in range(B):
        in_maps.append(
            {
                "seg16": seg16[i].reshape(H, NCLS * W),
                "nx": nx16[i].reshape(H, NF),
                "ny": ny16[i].reshape(H, NF),
                "wgt": w16[i].reshape(H, NF),
                "cwv": cwv,
                "chv": chv,
            }
        )

    from concourse.bass_utils import run_bass_kernel_spmd

    trace = bool(int(os.environ.get("KERNEL_TRACE", "0")))
    res = run_bass_kernel_spmd(
        nc, in_maps, core_ids=list(range(N_CORES)), trace=trace
    )
    kernel._last_exec_ns = res.exec_time_ns
    kernel._last_results = res

    out = np.stack(
        [_solve_host(np.asarray(res.results[i]["acc"])) for i in range(B)], axis=0
    )
    return out


# revision 34
# speedup vs baseline: 1.1444x; 1.0657x over previous
"""Trainium2 Bass kernel for CoordLSVotingWeighted (segment_reduce).

Strategy: data-parallel over batch B=8 across 8 NeuronCores (1 image/core).
Host prep (per image): de-interleave `direct` into unit nx/ny (bf16),
transpose w (bf16) and seg (fp16) to channel-major [H, C, W] layouts.
Device per image:
  - softplus(w) on ScalarE: sp = Ln(1 + Exp(w))
  - hard one-hot of argmax over 9 seg channels via DVE max-tree + is_equal
  - lhs features L = {hot, hot*ch, hot*cw} (bf16), rhs features
    F = {sp, R11=sp*nx^2, m=sp*nx*ny} (bf16; R00 = sp - R11 recovered on host)
  - segment reduce via 32 accumulating TensorE matmuls, 4 w-columns per
    matmul packed block-diagonally: lhsT [128, 4x32] (FWL-sized 128 cols),
    rhs [128, 4x27] -> PSUM [128, 108]; host sums the 4 diagonal blocks.
  - PE warm-up matmuls on junk data keep the HAM clock gate at full rate.
Host: assemble 2x2 systems in float64, pinv-solve, scale by HEIGHT.

Self-contained: only needs numpy / ml_dtypes / concourse (installed env).
"""

import os

import numpy as np

B = 8
H = 128
W = 128
NCLS = 9  # seg channels, class 0 = background
NPTS = 9
OC = 8
HEIGHT = 128.0
N_CORES = 8

NF = W * NPTS  # 1152
GJ = 4  # w-columns per matmul (block-diagonal packing)
LF = 32  # lhs feature rows (24 real + 8 pad) -> GJ*LF = 128 weight cols (FWL)
RF = 27  # rhs feature rows -> GJ*RF = 108 psum cols
N_MM = W // GJ  # 32 accumulating matmuls
N_WARM_A = 40  # junk matmuls during the DMA window (HAM clock ramp)

_cache: dict = {}


def _build_nc():
    import concourse.bacc as bacc
    import concourse.tile as tile
    import concourse.mybir as mybir
    from concourse.alu_op_type import AluOpType as Alu

    Act = mybir.ActivationFunctionType
    f32 = mybir.dt.float32
    f16 = mybir.dt.float16
    b16 = mybir.dt.bfloat16

    nc = bacc.Bacc(
        "TRN2", target_bir_lowering=False, debug=False, num_devices=N_CORES
    )
    seg_d = nc.dram_tensor("seg16", [H, NCLS * W], f16, kind="ExternalInput")
    nx_d = nc.dram_tensor("nx", [H, NF], b16, kind="ExternalInput")
    ny_d = nc.dram_tensor("ny", [H, NF], b16, kind="ExternalInput")
    w_d = nc.dram_tensor("wgt", [H, NF], b16, kind="ExternalInput")
    cw_d = nc.dram_tensor("cwv", [H, W + 1], b16, kind="ExternalInput")
    out_d = nc.dram_tensor("acc", [GJ * RF, GJ * LF], f32, kind="ExternalOutput")

    with tile.TileContext(nc) as tc:
        with (
            tc.tile_pool(name="main", bufs=1) as pool,
            tc.tile_pool(name="ps", bufs=1, space="PSUM") as psp,
        ):
            segt = pool.tile([H, NCLS * W], f16, tag="segt")
            nxt = pool.tile([H, NF], b16, tag="nxt")
            nyt = pool.tile([H, NF], b16, tag="nyt")
            wt = pool.tile([H, NF], b16, tag="wt")
            cwt = pool.tile([H, W + 1], b16, tag="cwt")
            cht = pool.tile([H, 1], f32, tag="cht")
            warm = pool.tile([H, W], b16, tag="warm")
            ew = pool.tile([H, NF], b16, tag="ew")
            ut = pool.tile([H, NF], b16, tag="ut")
            tmx = pool.tile([H, 8 * W], f16, tag="tmx")
            L = pool.tile([H, LF * W], b16, tag="L")
            R = pool.tile([H, RF * W], b16, tag="R")
            outs = pool.tile([GJ * RF, GJ * LF], f32, tag="outs")

            acc = psp.tile([GJ * RF, GJ * LF], f32, tag="acc")
            pwarm = psp.tile([GJ * RF, GJ * LF], f32, tag="pwarm")

            # ---- memsets first (PE warm-up depends on `warm`), then DMA
            # dispatches split across two queues so transfers start early.
            nc.gpsimd.memset(warm[:, :], 0.0)
            # zero the 8 pad feature rows of L (f = 24..31)
            nc.gpsimd.memset(L[:, 24 * W : 32 * W], 0.0)
            nc.sync.dma_start(out=wt[:, :], in_=w_d[:, :])
            nc.sync.dma_start(out=cwt[:, :], in_=cw_d[:, :])
            nc.gpsimd.dma_start(out=segt[:, :], in_=seg_d[:, :])
            nc.gpsimd.dma_start(out=nxt[:, :], in_=nx_d[:, :])
            nc.gpsimd.dma_start(out=nyt[:, :], in_=ny_d[:, :])
            # ch arrives as bf16 col W of cwv; upcast to f32 for tensor_scalar
            nc.vector.tensor_copy(out=cht[:, :], in_=cwt[:, W : W + 1])

            # ---- PE warm-up A: junk matmuls during the DMA window
            for _ in range(N_WARM_A):
                nc.tensor.matmul(
                    pwarm[:, :], warm[:, 0 : GJ * RF], warm[:, 0 : GJ * LF],
                    start=True, stop=True,
                )

            # ---- softplus on ScalarE: sp = Ln(1 + Exp(w)) -> R rows 0..8
            nc.scalar.activation(out=ew[:, :], in_=wt[:, :], func=Act.Exp)
            nc.scalar.activation(
                out=R[:, 0:NF], in_=ew[:, :], func=Act.Ln, bias=1.0
            )

            # ---- one-hot via DVE max-tree (channel-major fp16) + is_equal
            t1 = tmx[:, 0 : 4 * W]
            t2 = tmx[:, 4 * W : 6 * W]
            t3 = tmx[:, 6 * W : 7 * W]
            mx = tmx[:, 7 * W : 8 * W]
            nc.vector.tensor_tensor(
                out=t1, in0=segt[:, W : 5 * W], in1=segt[:, 5 * W : 9 * W],
                op=Alu.max,
            )
            nc.vector.tensor_tensor(
                out=t2, in0=t1[:, 0 : 2 * W], in1=t1[:, 2 * W : 4 * W], op=Alu.max
            )
            nc.vector.tensor_tensor(
                out=t3, in0=t2[:, 0:W], in1=t2[:, W : 2 * W], op=Alu.max
            )
            nc.vector.tensor_tensor(
                out=mx, in0=t3, in1=segt[:, 0:W], op=Alu.max
            )
            seg_fg = segt[:, W : 9 * W].rearrange("q (c w) -> q c w", c=OC)
            mx_b = mx.unsqueeze(1).broadcast_to((H, OC, W))
            hot_r = L[:, 0 : OC * W].rearrange("q (c w) -> q c w", c=OC)
            nc.vector.tensor_tensor(
                out=hot_r, in0=seg_fg, in1=mx_b, op=Alu.is_equal
            )
            # hot*ch (per-partition scalar), hot*cw (broadcast along c)
            nc.vector.tensor_scalar_mul(
                L[:, OC * W : 2 * OC * W], L[:, 0 : OC * W], cht[:, :]
            )
            cw_b = cwt[:, 0:W].unsqueeze(1).broadcast_to((H, OC, W))
            nc.vector.tensor_tensor(
                out=L[:, 2 * OC * W : 3 * OC * W].rearrange(
                    "q (c w) -> q c w", c=OC
                ),
                in0=L[:, 0 : OC * W].rearrange("q (c w) -> q c w", c=OC),
                in1=cw_b, op=Alu.mult,
            )

            # ---- PE warm-up B: staged junk matmuls gated on successive
            # producer outputs so the PE never idles long enough for the
            # HAM clock gate to re-throttle before the real matmuls.
            for _ in range(6):
                nc.tensor.matmul(
                    pwarm[:, :], R[:, 0 : GJ * RF], warm[:, 0 : GJ * LF],
                    start=True, stop=True,
                )

            # ---- rhs features: u = sp*nx, R11 = u*nx, m = u*ny
            nc.vector.tensor_tensor(
                out=ut[:, :], in0=R[:, 0:NF], in1=nxt[:, :], op=Alu.mult
            )
            for _ in range(6):
                nc.tensor.matmul(
                    pwarm[:, :], ut[:, 0 : GJ * RF], warm[:, 0 : GJ * LF],
                    start=True, stop=True,
                )
            nc.vector.tensor_tensor(
                out=R[:, NF : 2 * NF], in0=ut[:, :], in1=nxt[:, :], op=Alu.mult
            )
            for _ in range(4):
                nc.tensor.matmul(
                    pwarm[:, :], R[:, NF : NF + GJ * RF], warm[:, 0 : GJ * LF],
                    start=True, stop=True,
                )
            nc.vector.tensor_tensor(
                out=R[:, 2 * NF : 3 * NF], in0=ut[:, :], in1=nyt[:, :], op=Alu.mult
            )

            # ---- segment reduce: 32 accumulating matmuls, 4 w-cols each,
            # packed block-diagonally via flat stride-32 slices:
            # stationary R[:, i::32] cols c=4g+t <-> (g, w=i+32t), 108 cols;
            # moving L[:, i::32] cols c'=4f+t' <-> (f, w=i+32t'), 128 cols.
            # psum[c, c'] valid where t == t'; host sums the diagonal.
            for i in range(N_MM):
                nc.tensor.matmul(
                    acc[:, :],
                    R[:, i::N_MM],
                    L[:, i::N_MM],
                    start=(i == 0),
                    stop=(i == N_MM - 1),
                )

            nc.scalar.copy(out=outs[:, :], in_=acc[:, :])
            nc.sync.dma_start(out=out_d[:, :], in_=outs[:, :])

    nc.compile()

    # Post-compile: both Exp and Ln live in the natural_log_exp_and_others
    # ACT table set (index 6 in act_info.json), but the table-load pass
    # assigns each function its first containing set, inserting two loads.
    # Point the first load at the shared set and drop the rest (~1.3us each
    # off the critical path; walrus adopts pre-placed loads).
    NL_EXP_SET = 6
    loads = []
    for blk in nc.main_func.blocks:
        for ins in blk.instructions:
            if isinstance(ins, mybir.InstLoadActFuncSet):
                loads.append((blk, ins))
    assert len(loads) >= 1
    loads[0][1].act_func_set_id = NL_EXP_SET
    for blk, ins in loads[1:]:
        blk.instructions.remove(ins)
    return nc


def _host_inputs(seg, direct, w):
    import ml_dtypes

    bf16 = ml_dtypes.bfloat16
    # unit direction vectors (divide_no_nan semantics)
    n = direct.reshape(B, H, W, NPTS, 2).astype(np.float32)
    norm = np.sqrt(n[..., 0] ** 2 + n[..., 1] ** 2)
    safe = np.where(norm == 0.0, 1.0, norm)
    nx = np.where(norm == 0.0, 0.0, n[..., 0] / safe)
    ny = np.where(norm == 0.0, 0.0, n[..., 1] / safe)
    # [B, H, W, C] -> channel-major per-row [B, H, C, W] contiguous
    seg16 = np.ascontiguousarray(seg.transpose(0, 1, 3, 2)).astype(np.float16)
    nx16 = np.ascontiguousarray(nx.transpose(0, 1, 3, 2)).astype(bf16)
    ny16 = np.ascontiguousarray(ny.transpose(0, 1, 3, 2)).astype(bf16)
    w16 = np.ascontiguousarray(w.transpose(0, 1, 3, 2)).astype(bf16)
    # cwv: cols 0..W-1 = cw (same every row), col W = ch (per row).
    # (2h+1)/256 has <= 8 significant bits -> exact in bf16.
    coord = (np.arange(W, dtype=np.float32) + 0.5) / HEIGHT
    cwv = np.empty((H, W + 1), dtype=np.float32)
    cwv[:, 0:W] = coord[None, :]
    cwv[:, W] = coord
    return seg16, nx16, ny16, w16, cwv.astype(bf16)


def _solve_host(acc_raw: np.ndarray) -> np.ndarray:
    """acc [108,128] fp32 -> p [OC, NPTS, 2] fp32 (float64 pinv like ref)."""
    x = acc_raw.astype(np.float64).reshape(RF, GJ, LF, GJ)
    a = np.einsum("gtft->fg", x)  # sum the GJ diagonal blocks -> [32, 27]
    A_sp = a[0:OC, 0:NPTS]
    A_r11 = a[0:OC, NPTS : 2 * NPTS]
    A_m = a[0:OC, 2 * NPTS : 3 * NPTS]
    C_sp = a[OC : 2 * OC, 0:NPTS]
    C_r11 = a[OC : 2 * OC, NPTS : 2 * NPTS]
    C_m = a[OC : 2 * OC, 2 * NPTS : 3 * NPTS]
    W_r11 = a[2 * OC : 3 * OC, NPTS : 2 * NPTS]
    W_m = a[2 * OC : 3 * OC, 2 * NPTS : 3 * NPTS]
    Rm = np.empty((OC, NPTS, 2, 2), dtype=np.float64)
    Rm[..., 0, 0] = A_sp - A_r11
    Rm[..., 0, 1] = -A_m
    Rm[..., 1, 0] = -A_m
    Rm[..., 1, 1] = A_r11
    q = np.stack([(C_sp - C_r11) - W_m, W_r11 - C_m], axis=-1)
    Rp = np.linalg.pinv(Rm.reshape(-1, 2, 2)).reshape(Rm.shape)
    p = np.einsum("cpij,cpj->cpi", Rp, q) * HEIGHT
    return p.astype(np.float32)


def kernel(seg, direct, w):
    if "nc" not in _cache:
        _cache["nc"] = _build_nc()
    nc = _cache["nc"]

    seg = np.asarray(seg, dtype=np.float32)
    direct = np.ascontiguousarray(np.asarray(direct, dtype=np.float32))
    w = np.asarray(w, dtype=np.float32)
    seg16, nx16, ny16, w16, cwv = _host_inputs(seg, direct, w)

    in_maps = []
    for i in range(B):
        in_maps.append(
            {
                "seg16": seg16[i].reshape(H, NCLS * W),
                "nx": nx16[i].reshape(H, NF),
                "ny": ny16[i].reshape(H, NF),
                "wgt": w16[i].reshape(H, NF),
                "cwv": cwv,
            }
        )

    from concourse.bass_utils import run_bass_kernel_spmd

    trace = bool(int(os.environ.get("KERNEL_TRACE", "0")))
    res = run_bass_kernel_spmd(
        nc, in_maps, core_ids=list(range(N_CORES)), trace=trace
    )
    kernel._last_exec_ns = res.exec_time_ns
    kernel._last_results = res

    out = np.stack(
        [_solve_host(np.asarray(res.results[i]["acc"])) for i in range(B)], axis=0
    )
    return out


# revision 36
# speedup vs baseline: 1.1687x; 1.0212x over previous
"""Trainium2 Bass kernel for CoordLSVotingWeighted (segment_reduce).

Strategy: data-parallel over batch B=8 across 8 NeuronCores (1 image/core).
Host prep (per image): de-interleave `direct` into unit nx/ny (bf16),
transpose w (bf16) and seg (fp16) to channel-major [H, C, W] layouts.
Device per image, split into two w-interleaved halves (w mod 32 in 0..15
vs 16..31) so the first half's matmuls overlap the second half's
elementwise work:
  - softplus(w) on ScalarE: sp = Ln(1 + Exp(w)), single shared ACT table
  - hard one-hot of argmax over 9 seg channels via DVE max-tree + is_equal
  - lhs features L = {hot, hot*ch, hot*cw} (bf16), rhs features
    F = {sp, R11=sp*nx^2, m=sp*nx*ny} (bf16; R00 = sp - R11 on host)
  - segment reduce via 2x16 accumulating TensorE matmuls, 4 w-columns per
    matmul packed block-diagonally via flat stride-32 slices:
    stationary R[:, i::32] (108 cols), moving L[:, i::32] (128 cols),
    PSUM [108, 128]; host sums the 4 diagonal blocks.
  - junk matmuls staged on producer outputs keep the PE HAM clock warm.
Host: assemble 2x2 systems in float64, pinv-solve, scale by HEIGHT.

Self-contained: only needs numpy / ml_dtypes / concourse (installed env).
"""

import os

import numpy as np

B = 8
H = 128
W = 128
NCLS = 9  # seg channels, class 0 = background
NPTS = 9
OC = 8
HEIGHT = 128.0
N_CORES = 8

NF = W * NPTS  # 1152
GJ = 4  # w-columns per matmul (block-diagonal packing)
LF = 32  # lhs feature rows (24 real + 8 pad) -> GJ*LF = 128 moving cols
RF = 27  # rhs feature rows -> GJ*RF = 108 stationary cols
N_MM = W // GJ  # 32 accumulating matmuls (16 per half)
N_WARM_A = 40  # junk matmuls during the DMA window (HAM clock ramp)

_cache: dict = {}


def _build_nc():
    import concourse.bacc as bacc
    import concourse.tile as tile
    import concourse.mybir as mybir
    from concourse.alu_op_type import AluOpType as Alu

    Act = mybir.ActivationFunctionType
    f32 = mybir.dt.float32
    f16 = mybir.dt.float16
    b16 = mybir.dt.bfloat16

    nc = bacc.Bacc(
        "TRN2", target_bir_lowering=False, debug=False, num_devices=N_CORES
    )
    seg_d = nc.dram_tensor("seg16", [H, NCLS * W], f16, kind="ExternalInput")
    nx_d = nc.dram_tensor("nx", [H, NF], b16, kind="ExternalInput")
    ny_d = nc.dram_tensor("ny", [H, NF], b16, kind="ExternalInput")
    w_d = nc.dram_tensor("wgt", [H, NF], b16, kind="ExternalInput")
    cw_d = nc.dram_tensor("cwv", [H, W + 1], b16, kind="ExternalInput")
    out_d = nc.dram_tensor("acc", [GJ * RF, 2 * GJ * LF], f32, kind="ExternalOutput")

    def hv(flat, F, h):
        """Half-h view [q, F, 4, 16] of a feature-major [H, F*128] region."""
        return flat.rearrange("q (f b v) -> q f b v", f=F, b=8)[:, :, h::2, :]

    with tile.TileContext(nc) as tc:
        with (
            tc.tile_pool(name="main", bufs=1) as pool,
            tc.tile_pool(name="ps", bufs=1, space="PSUM") as psp,
        ):
            segt = pool.tile([H, NCLS * W], f16, tag="segt")
            nxt = pool.tile([H, NF], b16, tag="nxt")
            nyt = pool.tile([H, NF], b16, tag="nyt")
            wt = pool.tile([H, NF], b16, tag="wt")
            cwt = pool.tile([H, W + 1], b16, tag="cwt")
            cht = pool.tile([H, 1], f32, tag="cht")
            warm = pool.tile([H, W], b16, tag="warm")
            ew = pool.tile([H, NF], b16, tag="ew")
            ut = pool.tile([H, NF], b16, tag="ut")
            tmx = pool.tile([H, 8 * W], f16, tag="tmx")
            L = pool.tile([H, LF * W], b16, tag="L")
            R = pool.tile([H, RF * W], b16, tag="R")
            outs = pool.tile([GJ * RF, 2 * GJ * LF], f32, tag="outs")

            acc0 = psp.tile([GJ * RF, GJ * LF], f32, tag="acc0")
            acc1 = psp.tile([GJ * RF, GJ * LF], f32, tag="acc1")
            accs = [acc0, acc1]
            pwarm = psp.tile([GJ * RF, GJ * LF], f32, tag="pwarm")

            # ---- memsets first (PE warm-up depends on `warm`), then DMA
            # dispatches split across two queues so transfers start early.
            nc.gpsimd.memset(warm[:, :], 0.0)
            # zero the 8 pad feature rows of L (f = 24..31)
            nc.gpsimd.memset(L[:, 24 * W : 32 * W], 0.0)
            nc.sync.dma_start(out=wt[:, :], in_=w_d[:, :])
            nc.sync.dma_start(out=cwt[:, :], in_=cw_d[:, :])
            nc.gpsimd.dma_start(out=segt[:, :], in_=seg_d[:, :])
            nc.gpsimd.dma_start(out=nxt[:, :], in_=nx_d[:, :])
            nc.gpsimd.dma_start(out=nyt[:, :], in_=ny_d[:, :])
            # ch arrives as bf16 col W of cwv; upcast to f32 for tensor_scalar
            nc.vector.tensor_copy(out=cht[:, :], in_=cwt[:, W : W + 1])

            # ---- PE warm-up A: junk matmuls during the DMA window
            for _ in range(N_WARM_A):
                nc.tensor.matmul(
                    pwarm[:, :], warm[:, 0 : GJ * RF], warm[:, 0 : GJ * LF],
                    start=True, stop=True,
                )

            # ---- softplus on ScalarE: sp = Ln(1 + Exp(w)) -> R rows 0..8
            # (half A first so the DVE feature chain can start early)
            for h in (0, 1):
                nc.scalar.activation(
                    out=hv(ew[:, :], NPTS, h), in_=hv(wt[:, :], NPTS, h),
                    func=Act.Exp,
                )
                nc.scalar.activation(
                    out=hv(R[:, :], RF, h)[:, 0:NPTS],
                    in_=hv(ew[:, :], NPTS, h), func=Act.Ln, bias=1.0,
                )

            # ---- per-half DVE chains + staged PE warm-ups + real matmuls
            cw_v = cwt[:, 0:W].rearrange("q (b v) -> q b v", b=8)
            for h in (0, 1):
                base = h * 4 * W
                t1 = tmx[:, base : base + 256].rearrange(
                    "q (c b v) -> q c b v", c=4, b=4
                )
                t2 = tmx[:, base + 256 : base + 384].rearrange(
                    "q (c b v) -> q c b v", c=2, b=4
                )
                t3 = tmx[:, base + 384 : base + 448].rearrange(
                    "q (c b v) -> q c b v", c=1, b=4
                )
                mxf = tmx[:, base + 448 : base + 512]
                mx = mxf.rearrange("q (c b v) -> q c b v", c=1, b=4)
                sg = hv(segt[:, :], NCLS, h)
                Lh = hv(L[:, :], LF, h)
                Rh = hv(R[:, :], RF, h)

                nc.vector.tensor_tensor(
                    out=t1, in0=sg[:, 1:5], in1=sg[:, 5:9], op=Alu.max
                )
                nc.vector.tensor_tensor(
                    out=t2, in0=t1[:, 0:2], in1=t1[:, 2:4], op=Alu.max
                )
                nc.vector.tensor_tensor(
                    out=t3, in0=t2[:, 0:1], in1=t2[:, 1:2], op=Alu.max
                )
                nc.vector.tensor_tensor(
                    out=mx, in0=t3, in1=sg[:, 0:1], op=Alu.max
                )
                mx_b = (
                    mxf.rearrange("q (b v) -> q b v", b=4)
                    .unsqueeze(1)
                    .broadcast_to((H, OC, 4, 16))
                )
                nc.vector.tensor_tensor(
                    out=Lh[:, 0:OC], in0=sg[:, 1:NCLS], in1=mx_b, op=Alu.is_equal
                )
                nc.vector.tensor_scalar_mul(
                    Lh[:, OC : 2 * OC], Lh[:, 0:OC], cht[:, :]
                )
                cw_b = (
                    cw_v[:, h::2, :].unsqueeze(1).broadcast_to((H, OC, 4, 16))
                )
                nc.vector.tensor_tensor(
                    out=Lh[:, 2 * OC : 3 * OC], in0=Lh[:, 0:OC], in1=cw_b,
                    op=Alu.mult,
                )

                # staged junk matmuls (regions inside half h only) keep the
                # PE HAM clock warm across the DVE chain
                if h == 0:
                    for _ in range(10):
                        nc.tensor.matmul(
                            pwarm[0:16, :], R[:, 0:16], warm[:, 0 : GJ * LF],
                            start=True, stop=True,
                        )

                # rhs features: u = sp*nx, R11 = u*nx, m = u*ny
                uh = hv(ut[:, :], NPTS, h)
                nxh = hv(nxt[:, :], NPTS, h)
                nyh = hv(nyt[:, :], NPTS, h)
                nc.vector.tensor_tensor(
                    out=uh, in0=Rh[:, 0:NPTS], in1=nxh, op=Alu.mult
                )
                if h == 0:
                    for _ in range(6):
                        nc.tensor.matmul(
                            pwarm[0:16, :], ut[:, 0:16], warm[:, 0 : GJ * LF],
                            start=True, stop=True,
                        )
                nc.vector.tensor_tensor(
                    out=Rh[:, NPTS : 2 * NPTS], in0=uh, in1=nxh, op=Alu.mult
                )
                if h == 0:
                    for _ in range(4):
                        nc.tensor.matmul(
                            pwarm[0:16, :], R[:, NF : NF + 16],
                            warm[:, 0 : GJ * LF], start=True, stop=True,
                        )
                nc.vector.tensor_tensor(
                    out=Rh[:, 2 * NPTS : 3 * NPTS], in0=uh, in1=nyh, op=Alu.mult
                )

                # ---- segment reduce for half h: 16 accumulating matmuls,
                # 4 w-cols each, packed block-diagonally via flat stride-32
                # slices: stationary R[:, i::32] cols c=4g+t <-> (g, w=i+32t);
                # moving L[:, i::32] cols c'=4f+t' <-> (f, w=i+32t').
                # psum[c, c'] valid where t == t'; host sums the diagonal.
                for j in range(16):
                    i = 16 * h + j
                    nc.tensor.matmul(
                        accs[h][:, :],
                        R[:, i::N_MM],
                        L[:, i::N_MM],
                        start=(j == 0),
                        stop=(j == 15),
                    )
                nc.scalar.copy(
                    out=outs[:, h * GJ * LF : (h + 1) * GJ * LF],
                    in_=accs[h][:, :],
                )

            nc.sync.dma_start(out=out_d[:, :], in_=outs[:, :])

    nc.compile()

    # Post-compile: both Exp and Ln live in the natural_log_exp_and_others
    # ACT table set (index 6 in act_info.json), but the table-load pass
    # assigns each function its first containing set, inserting a load per
    # switch. Point the first load at the shared set and drop the rest
    # (~1.3us each off the critical path; walrus adopts pre-placed loads).
    NL_EXP_SET = 6
    loads = []
    for blk in nc.main_func.blocks:
        for ins in blk.instructions:
            if isinstance(ins, mybir.InstLoadActFuncSet):
                loads.append((blk, ins))
    assert len(loads) >= 1
    loads[0][1].act_func_set_id = NL_EXP_SET
    for blk, ins in loads[1:]:
        blk.instructions.remove(ins)
    return nc


def _host_inputs(seg, direct, w):
    import ml_dtypes

    bf16 = ml_dtypes.bfloat16
    # unit direction vectors (divide_no_nan semantics)
    n = direct.reshape(B, H, W, NPTS, 2).astype(np.float32)
    norm = np.sqrt(n[..., 0] ** 2 + n[..., 1] ** 2)
    safe = np.where(norm == 0.0, 1.0, norm)
    nx = np.where(norm == 0.0, 0.0, n[..., 0] / safe)
    ny = np.where(norm == 0.0, 0.0, n[..., 1] / safe)
    # [B, H, W, C] -> channel-major per-row [B, H, C, W] contiguous
    seg16 = np.ascontiguousarray(seg.transpose(0, 1, 3, 2)).astype(np.float16)
    nx16 = np.ascontiguousarray(nx.transpose(0, 1, 3, 2)).astype(bf16)
    ny16 = np.ascontiguousarray(ny.transpose(0, 1, 3, 2)).astype(bf16)
    w16 = np.ascontiguousarray(w.transpose(0, 1, 3, 2)).astype(bf16)
    # cwv: cols 0..W-1 = cw (same every row), col W = ch (per row).
    # (2h+1)/256 has <= 8 significant bits -> exact in bf16.
    coord = (np.arange(W, dtype=np.float32) + 0.5) / HEIGHT
    cwv = np.empty((H, W + 1), dtype=np.float32)
    cwv[:, 0:W] = coord[None, :]
    cwv[:, W] = coord
    return seg16, nx16, ny16, w16, cwv.astype(bf16)


def _solve_host(acc_raw: np.ndarray) -> np.ndarray:
    """acc [108,256] fp32 -> p [OC, NPTS, 2] fp32 (float64 pinv like ref)."""
    both = acc_raw.astype(np.float64)
    x = (both[:, 0 : GJ * LF] + both[:, GJ * LF :]).reshape(RF, GJ, LF, GJ)
    a = np.einsum("gtft->fg", x)  # sum the GJ diagonal blocks -> [32, 27]
    A_sp = a[0:OC, 0:NPTS]
    A_r11 = a[0:OC, NPTS : 2 * NPTS]
    A_m = a[0:OC, 2 * NPTS : 3 * NPTS]
    C_sp = a[OC : 2 * OC, 0:NPTS]
    C_r11 = a[OC : 2 * OC, NPTS : 2 * NPTS]
    C_m = a[OC : 2 * OC, 2 * NPTS : 3 * NPTS]
    W_r11 = a[2 * OC : 3 * OC, NPTS : 2 * NPTS]
    W_m = a[2 * OC : 3 * OC, 2 * NPTS : 3 * NPTS]
    Rm = np.empty((OC, NPTS, 2, 2), dtype=np.float64)
    Rm[..., 0, 0] = A_sp - A_r11
    Rm[..., 0, 1] = -A_m
    Rm[..., 1, 0] = -A_m
    Rm[..., 1, 1] = A_r11
    q = np.stack([(C_sp - C_r11) - W_m, W_r11 - C_m], axis=-1)
    Rp = np.linalg.pinv(Rm.reshape(-1, 2, 2)).reshape(Rm.shape)
    p = np.einsum("cpij,cpj->cpi", Rp, q) * HEIGHT
    return p.astype(np.float32)


def kernel(seg, direct, w):
    if "nc" not in _cache:
        _cache["nc"] = _build_nc()
    nc = _cache["nc"]

    seg = np.asarray(seg, dtype=np.float32)
    direct = np.ascontiguousarray(np.asarray(direct, dtype=np.float32))
    w = np.asarray(w, dtype=np.float32)
    seg16, nx16, ny16, w16, cwv = _host_inputs(seg, direct, w)

    in_maps = []
    for i in range(B):
        in_maps.append(
            {
                "seg16": seg16[i].reshape(H, NCLS * W),
                "nx": nx16[i].reshape(H, NF),
                "ny": ny16[i].reshape(H, NF),
                "wgt": w16[i].reshape(H, NF),
                "cwv": cwv,
            }
        )

    from concourse.bass_utils import run_bass_kernel_spmd

    trace = bool(int(os.environ.get("KERNEL_TRACE", "0")))
    res = run_bass_kernel_spmd(
        nc, in_maps, core_ids=list(range(N_CORES)), trace=trace
    )
    kernel._last_exec_ns = res.exec_time_ns
    kernel._last_results = res

    out = np.stack(
        [_solve_host(np.asarray(res.results[i]["acc"])) for i in range(B)], axis=0
    )
    return out
